# revision 2
# baseline (speedup 1.0000x reference)
"""Chamfer distance kernel for Trainium2 (8 NeuronCores).

Problem: pred/target [4, 8192, 3] f32 -> scalar
  mean_b( mean_m min_n ||p_bm - q_bn||^2 + mean_n min_m ||p_bm - q_bn||^2 )

Strategy (one "side" per core; 4 batches x 2 directions = 8 cores):
  Each core owns one (batch, direction) pair and computes, for each of its
  8192 "own" points, the min squared distance to all 8192 "other" points.

  Distances are produced on the TensorEngine as a K=8 matmul using the
  identity ||p-q||^2 = -2 p.q + ||p||^2 + ||q||^2:
      lhsT rows: [-2x, -2y, -2z, n_hi, n_lo, 1, 1, 0]   (own points)
      rhs  rows: [ x,   y,  z,  1,    1,  n_hi, n_lo, 0] (other points)
  Inputs are fp16 (1 cycle/row on PE); norms are split hi/lo into two fp16
  values so the norm contribution keeps ~fp29 precision; products of fp16
  are exact in the fp32 PSUM accumulation.

  The min-reduction runs on the VectorEngine via tensor_tensor_scan
  (op0=min, op1=min), which consumes TWO 1024-wide blocks per instruction
  (one directly from PSUM, one staged to SBUF by the ScalarEngine), i.e.
  0.5 DVE cycles per distance. Scans chain across an m-tile through the
  `initial` operand; the final scan column is that m-tile's row-min.
"""

import numpy as np

import concourse.bacc as bacc
import concourse.mybir as mybir
import concourse.tile as tile
from concourse import bass_utils

P = 128          # partitions / m-tile size
NPTS = 8192      # points per cloud
B = 4            # batch
K = 8            # matmul contraction (padded)
MT = NPTS // P   # 64 m-tiles
BLK = 1024       # free-dim block per scan operand (2 PSUM banks)
NBLK = NPTS // BLK  # 8 blocks per m-tile
MM_N = 512       # matmul free dim (one PSUM bank of fp32)

F16 = mybir.dt.float16
F32 = mybir.dt.float32
MIN = mybir.AluOpType.min
INF = 3.0e38


def _build_nc():
    nc = bacc.Bacc(
        "TRN2", target_bir_lowering=False, debug=False, num_devices=8
    )
    lhsT_d = nc.dram_tensor("lhsT", [K, NPTS], F16, kind="ExternalInput")
    rhs_d = nc.dram_tensor("rhs", [K, NPTS], F16, kind="ExternalInput")
    mins_d = nc.dram_tensor("mins", [P, MT], F32, kind="ExternalOutput")

    with tile.TileContext(nc) as tc:
        with (
            tc.tile_pool(name="const", bufs=1) as const,
            tc.tile_pool(name="psum", bufs=4, space="PSUM") as psum,
            tc.tile_pool(name="staged", bufs=3) as stg,
            tc.tile_pool(name="scratch", bufs=2) as scr,
        ):
            lt = const.tile([K, NPTS], F16)
            rt = const.tile([K, NPTS], F16)
            res = const.tile([P, MT], F32)
            nc.sync.dma_start(lt[:], lhsT_d.ap())
            nc.sync.dma_start(rt[:], rhs_d.ap())

            for t in range(MT):
                lslice = lt[:, t * P:(t + 1) * P]
                chain = None
                st_prev = None
                for blk in range(NBLK):
                    ps = psum.tile([P, BLK], F32)
                    for j in range(BLK // MM_N):
                        n0 = blk * BLK + j * MM_N
                        nc.tensor.matmul(
                            ps[:, j * MM_N:(j + 1) * MM_N],
                            lslice,
                            rt[:, n0:n0 + MM_N],
                            start=True,
                            stop=True,
                        )
                    if blk % 2 == 0:
                        st = stg.tile([P, BLK], F32)
                        nc.scalar.copy(st[:], ps[:])
                        st_prev = st
                    else:
                        sc = scr.tile([P, BLK], F32)
                        init = INF if chain is None else chain
                        nc.vector.tensor_tensor_scan(
                            sc[:], ps[:], st_prev[:], init, op0=MIN, op1=MIN
                        )
                        chain = sc[:, BLK - 1:BLK]
                nc.scalar.copy(res[:, t:t + 1], chain)

            nc.sync.dma_start(mins_d.ap(), res[:])

    nc.compile()
    return nc


_NC_CACHE = []


def _get_nc():
    if not _NC_CACHE:
        _NC_CACHE.append(_build_nc())
    return _NC_CACHE[0]


def _prep_side(own, other):
    """Build lhsT [8, N] fp16 (own) and rhs [8, N] fp16 (other)."""
    o16 = own.astype(np.float16)
    t16 = other.astype(np.float16)
    o32 = o16.astype(np.float32)
    t32 = t16.astype(np.float32)
    on = (o32 * o32).sum(-1)       # fp32 norms of the fp16-rounded points
    tn = (t32 * t32).sum(-1)
    on_hi = on.astype(np.float16)
    on_lo = (on - on_hi.astype(np.float32)).astype(np.float16)
    tn_hi = tn.astype(np.float16)
    tn_lo = (tn - tn_hi.astype(np.float32)).astype(np.float16)

    n = own.shape[0]
    lhsT = np.zeros((K, n), np.float16)
    lhsT[0:3] = (-2.0 * o32).astype(np.float16).T
    lhsT[3] = on_hi
    lhsT[4] = on_lo
    lhsT[5] = 1.0
    lhsT[6] = 1.0
    rhs = np.zeros((K, n), np.float16)
    rhs[0:3] = t16.T
    rhs[3] = 1.0
    rhs[4] = 1.0
    rhs[5] = tn_hi
    rhs[6] = tn_lo
    return lhsT, rhs


def _in_maps_for(pred, target):
    pred = np.asarray(pred, dtype=np.float32)
    target = np.asarray(target, dtype=np.float32)
    in_maps = []
    for b in range(B):
        for d in range(2):
            own, other = (
                (pred[b], target[b]) if d == 0 else (target[b], pred[b])
            )
            lhsT, rhs = _prep_side(own, other)
            in_maps.append({"lhsT": lhsT, "rhs": rhs})
    return in_maps


def kernel(pred, target):
    in_maps = _in_maps_for(pred, target)
    nc = _get_nc()
    r = bass_utils.run_bass_kernel_spmd(nc, in_maps, core_ids=list(range(8)))

    total = 0.0
    for core_res in r.results:
        total += core_res["mins"].astype(np.float64).mean()
    return np.array(total / B, dtype=np.float32)


# revision 3
# speedup vs baseline: 1.2028x; 1.2028x over previous
"""Chamfer distance kernel for Trainium2 (8 NeuronCores).

Problem: pred/target [4, 8192, 3] f32 -> scalar
  mean_b( mean_m min_n ||p_bm - q_bn||^2 + mean_n min_m ||p_bm - q_bn||^2 )

Strategy (one "side" per core; 4 batches x 2 directions = 8 cores):
  Each core owns one (batch, direction) pair and computes, for each of its
  8192 "own" points, the min squared distance to all 8192 "other" points.

  Distances are produced on the TensorEngine as a K=8 matmul using the
  identity ||p-q||^2 = -2 p.q + ||p||^2 + ||q||^2:
      lhsT rows: [-2x, -2y, -2z, n_hi, n_lo, 1, 1, 0]   (own points)
      rhs  rows: [ x,   y,  z,  1,    1,  n_hi, n_lo, 0] (other points)
  Inputs are fp16 (1 cycle/row on PE); norms are split hi/lo into two fp16
  values so the norm contribution keeps ~2^-22 precision; fp16 products are
  exact in the fp32 PSUM accumulation.

  The min-reduction is a pair-min tree sized from measured HW op costs:
    - ScalarE drains most PSUM blocks to fp16 SBUF (ACTIVATE copy, 1 elem/cyc)
    - VectorE L1 collapses pairs 2:1 with tensor_tensor(min):
        (PSUM fp32, staged fp16) pairs run at 1x (both read ports busy),
        (staged, staged) fp16 pairs run in the 2x_1p DVE mode
    - VectorE ladder merges L1 outputs (fp16 2x) and a final
      tensor_reduce(min) yields the per-m-tile row minima.
  The drained/direct mix alternates per m-tile to balance ScalarE vs
  VectorE load (both engines end up ~equally busy).
"""

import numpy as np

import concourse.bacc as bacc
import concourse.mybir as mybir
import concourse.tile as tile
from concourse import bass_utils

P = 128          # partitions / m-tile size
NPTS = 8192      # points per cloud
B = 4            # batch
K = 8            # matmul contraction (padded)
MT = NPTS // P   # 64 m-tiles
BLK = 1024       # block width (2 PSUM banks)
NBLK = NPTS // BLK  # 8 blocks per m-tile
MM_N = 512       # matmul free dim (one PSUM bank of fp32)

F16 = mybir.dt.float16
F32 = mybir.dt.float32
MIN = mybir.AluOpType.min


def _emit_mtile(nc, psum, stg, xpool, ypool, lslice, rt, res_col, n_direct):
    """One m-tile: 8 blocks of 1024 distances, reduced to [128,1] row-mins.

    n_direct PSUM blocks are consumed directly by VectorE L1 pair-mins;
    the remaining 8-n_direct blocks are staged to fp16 SBUF by ScalarE.
    """
    n_staged = NBLK - n_direct
    # matmul all 8 blocks into rotating PSUM tiles; stage the first
    # n_staged, pair the last n_direct against staged partners.
    staged = []
    direct_ps = []
    for blk in range(NBLK):
        ps = psum.tile([P, BLK], F32, tag="ps")
        for j in range(BLK // MM_N):
            n0 = blk * BLK + j * MM_N
            nc.tensor.matmul(
                ps[:, j * MM_N:(j + 1) * MM_N],
                lslice,
                rt[:, n0:n0 + MM_N],
                start=True,
                stop=True,
            )
        if blk < n_staged:
            st = stg.tile([P, BLK], F16, tag="st")
            nc.scalar.copy(st[:], ps[:])
            staged.append(st)
        else:
            direct_ps.append(ps)

    # L1: collapse to 4 fp16 blocks.
    xs = []
    si = 0
    # staged-staged pairs first (fp16 2x mode)
    n_ss_pairs = (n_staged - n_direct) // 2
    for _ in range(n_ss_pairs):
        x = xpool.tile([P, BLK], F16, tag="x")
        nc.vector.tensor_tensor(x[:], staged[si][:], staged[si + 1][:], op=MIN)
        xs.append(x)
        si += 2
    # psum-staged pairs (1x, drains PSUM on the DVE port)
    for ps in direct_ps:
        x = xpool.tile([P, BLK], F16, tag="x")
        nc.vector.tensor_tensor(x[:], ps[:], staged[si][:], op=MIN)
        xs.append(x)
        si += 1
    assert si == n_staged and len(xs) == 4

    # L2/L3 ladder (fp16 2x) down to one block.
    y0 = ypool.tile([P, BLK], F16, tag="y")
    nc.vector.tensor_tensor(y0[:], xs[0][:], xs[1][:], op=MIN)
    y1 = ypool.tile([P, BLK], F16, tag="y")
    nc.vector.tensor_tensor(y1[:], xs[2][:], xs[3][:], op=MIN)
    z = xpool.tile([P, BLK], F16, tag="z")
    nc.vector.tensor_tensor(z[:], y0[:], y1[:], op=MIN)
    v = ypool.tile([P, BLK // 2], F16, tag="v")
    nc.vector.tensor_tensor(v[:], z[:, :BLK // 2], z[:, BLK // 2:], op=MIN)
    nc.vector.tensor_reduce(res_col, v[:], axis=mybir.AxisListType.X, op=MIN)


def _build_nc():
    nc = bacc.Bacc(
        "TRN2", target_bir_lowering=False, debug=False, num_devices=8
    )
    lhsT_d = nc.dram_tensor("lhsT", [K, NPTS], F16, kind="ExternalInput")
    rhs_d = nc.dram_tensor("rhs", [K, NPTS], F16, kind="ExternalInput")
    mins_d = nc.dram_tensor("mins", [P, MT], F32, kind="ExternalOutput")

    with tile.TileContext(nc) as tc:
        with (
            tc.tile_pool(name="const", bufs=1) as const,
            tc.tile_pool(name="psum", bufs=4, space="PSUM") as psum,
            tc.tile_pool(name="stg", bufs=8) as stg,
            tc.tile_pool(name="xpool", bufs=3) as xpool,
            tc.tile_pool(name="ypool", bufs=3) as ypool,
        ):
            lt = const.tile([K, NPTS], F16)
            rt = const.tile([K, NPTS], F16)
            res = const.tile([P, MT], F32)
            nc.sync.dma_start(lt[:], lhsT_d.ap())
            nc.sync.dma_start(rt[:], rhs_d.ap())

            for t in range(MT):
                lslice = lt[:, t * P:(t + 1) * P]
                n_direct = 2 if t % 2 == 0 else 3
                _emit_mtile(
                    nc, psum, stg, xpool, ypool, lslice, rt,
                    res[:, t:t + 1], n_direct,
                )

            nc.sync.dma_start(mins_d.ap(), res[:])

    nc.compile()
    return nc


_NC_CACHE = []


def _get_nc():
    if not _NC_CACHE:
        _NC_CACHE.append(_build_nc())
    return _NC_CACHE[0]


def _prep_side(own, other):
    """Build lhsT [8, N] fp16 (own) and rhs [8, N] fp16 (other)."""
    o16 = own.astype(np.float16)
    t16 = other.astype(np.float16)
    o32 = o16.astype(np.float32)
    t32 = t16.astype(np.float32)
    on = (o32 * o32).sum(-1)       # fp32 norms of the fp16-rounded points
    tn = (t32 * t32).sum(-1)
    on_hi = on.astype(np.float16)
    on_lo = (on - on_hi.astype(np.float32)).astype(np.float16)
    tn_hi = tn.astype(np.float16)
    tn_lo = (tn - tn_hi.astype(np.float32)).astype(np.float16)

    n = own.shape[0]
    lhsT = np.zeros((K, n), np.float16)
    lhsT[0:3] = (-2.0 * o32).astype(np.float16).T
    lhsT[3] = on_hi
    lhsT[4] = on_lo
    lhsT[5] = 1.0
    lhsT[6] = 1.0
    rhs = np.zeros((K, n), np.float16)
    rhs[0:3] = t16.T
    rhs[3] = 1.0
    rhs[4] = 1.0
    rhs[5] = tn_hi
    rhs[6] = tn_lo
    return lhsT, rhs


def _in_maps_for(pred, target):
    pred = np.asarray(pred, dtype=np.float32)
    target = np.asarray(target, dtype=np.float32)
    in_maps = []
    for b in range(B):
        for d in range(2):
            own, other = (
                (pred[b], target[b]) if d == 0 else (target[b], pred[b])
            )
            lhsT, rhs = _prep_side(own, other)
            in_maps.append({"lhsT": lhsT, "rhs": rhs})
    return in_maps


def kernel(pred, target):
    in_maps = _in_maps_for(pred, target)
    nc = _get_nc()
    r = bass_utils.run_bass_kernel_spmd(nc, in_maps, core_ids=list(range(8)))

    total = 0.0
    for core_res in r.results:
        total += core_res["mins"].astype(np.float64).mean()
    return np.array(total / B, dtype=np.float32)
